# revision 6
# baseline (speedup 1.0000x reference)
"""Bass/Trainium2 kernel for nn_Attn_81690277970335.

reference:  proj = enc @ W.T + b        [S, H]
            energies = proj @ hidden    [S]
            attn = softmax(energies)    [1, 1, S]

Key algebraic identity (exact in exact arithmetic):
            energies = enc @ (W.T @ hidden) + (b . hidden)
and softmax is invariant to the constant shift (b . hidden).  So on device we
compute v = W.T @ hidden once (tiny matvec) and then a single [S,H] @ [H]
matvec over the big tensor -> memory-bound streaming of enc.

Distribution: encoder_outputs sharded along seq across 8 cores.  W is
row-split: core c loads only rows [128c, 128c+128) of W (0.5 MB instead of
4.2 MB), computes its partial v = sum_{o in slice} hidden[o] * W[o, :] on the
PE, and a tiny AllReduce(add) combines the partials -> full v on every core.
That AllReduce doubles as the CC-stream warm-up.

enc streaming: the whole 16.8 MB shard fits in SBUF (131 KB/partition), so
all 8 x 2MB chunk DMAs are issued up-front on the two HWDGE rings with
enough buffers that no DMA ever waits on compute.  Mid-kernel small DMAs
(collective bounce buffers) ride SWDGE (gpsimd) so they never queue behind
the enc stream in a HWDGE FIFO.

Dot products: fused multiply+reduce, split DVE (tensor_tensor_reduce, rows
0-1 of each chunk) / GpSimd (scalar_tensor_tensor + accum, rows 2-3), one
pass over the data per engine, ACT left free for the softmax tail.

Softmax: per-partition max m_p and sum s_p (exact, no cross-partition work
before the collective); one AllGather of the packed [2,128] (-m_p, s_p)
stats; each core redundantly combines all 8*128 pairs to the global (M, sum)
and rescales its local exp(e - m_p) tile.
"""

import sys

sys.path.insert(0, "/opt/trn_rl_repo")

import numpy as np

import concourse.bass as bass
import concourse.mybir as mybir
import concourse.tile as tile
from concourse.bass_utils import run_bass_kernel_spmd

SEQ = 32768
HID = 1024
NCORES = 8
SHARD = SEQ // NCORES  # 4096
P = 128  # partitions
TW = SHARD // P  # 32 seq rows per partition
CU = 4  # rows per chunk (per partition)
NCH = TW // CU  # 8 chunks
F32 = mybir.dt.float32
AL = mybir.AluOpType
ACT = mybir.ActivationFunctionType

_CACHE = {}


def _split_multiwaits(nc):
    """This container's walrus build accepts at most ONE sync-wait per
    instruction; Tile emits several.  Hoist extra waits onto single-wait
    NoOps inserted just before the instruction on the same engine queue
    (engines and DGE-issuing sequencers are in-order, so semantics hold)."""
    import bass_rust

    cnt = 0
    for f in nc.m.functions:
        for bb in f.blocks:
            il = bb.instructions
            i = 0
            while i < len(il):
                inst = il[i]
                si = inst.sync_info
                if si is not None and si.on_wait and len(si.on_wait) > 1:
                    waits = list(si.on_wait)
                    keep, extra = waits[-1], waits[:-1]
                    for j, w in enumerate(extra):
                        nop = mybir.InstNoOp(
                            name=f"{inst.name}-w{j}", ins=[], outs=[]
                        )
                        nop.engine = inst.engine
                        nop.sync_info = bass_rust.SyncInfo(
                            on_wait=[w], on_update=[]
                        )
                        il.insert(i, nop)
                        i += 1
                        cnt += 1
                    inst.sync_info = bass_rust.SyncInfo(
                        on_wait=[keep], on_update=list(si.on_update or [])
                    )
                i += 1
    return cnt


def _build_nc():
    nc = bass.Bass(num_devices=NCORES)

    enc = nc.dram_tensor("enc", [SHARD, HID], F32, kind="ExternalInput")
    # per-core W row-slice: rows [128c, 128c+128) of W -> [128 o, 1024 h]
    wr = nc.dram_tensor("wr", [P, HID], F32, kind="ExternalInput")
    # aux: [128, 1 + 128 + 128]: col 0 = hidden[128c + p]; ident; ones
    AUXW = 1 + P + P
    aux = nc.dram_tensor("aux", [P, AUXW], F32, kind="ExternalInput")
    out = nc.dram_tensor("attn", [SHARD], F32, kind="ExternalOutput")

    # collective bounce buffers (internal DRAM; outputs must be Shared)
    ccv_in = nc.dram_tensor("ccv_in", [1, HID], F32)
    ccv_out = nc.dram_tensor("ccv_out", [1, HID], F32, addr_space="Shared")
    cc_in = nc.dram_tensor("cc_in", [2, P], F32)
    cc_out = nc.dram_tensor("cc_out", [2 * NCORES, P], F32, addr_space="Shared")

    # seq row s of the shard lives at (partition p, column t): s = p*TW + t,
    # t = c*CU + u.  Per chunk c: 4 consecutive rows -> 16KB contiguous per
    # partition -> line-rate descriptors.
    enc_v = enc.rearrange("(p c u) h -> c p u h", c=NCH, u=CU)  # [8,128,4,HID]
    out_v = out.rearrange("(p t) -> p t", t=TW)  # [128, TW]

    with tile.TileContext(nc) as tc:
        with (
            tc.tile_pool(name="wpool", bufs=1) as wpool,
            tc.tile_pool(name="encp", bufs=NCH) as encp,
            tc.tile_pool(name="jd", bufs=2) as jdp,
            tc.tile_pool(name="jg", bufs=2) as jgp,
            tc.tile_pool(name="small", bufs=1) as small,
            tc.tile_pool(name="ps_a", bufs=1, space="PSUM") as ps_a,
            tc.tile_pool(name="ps_b", bufs=1, space="PSUM") as ps_b,
            tc.tile_pool(name="ps_c", bufs=1, space="PSUM") as ps_c,
        ):
            # ---- head-of-ring DMAs: aux on sync, W rows on scalar ----------
            aux_sb = wpool.tile([P, AUXW], F32, tag="aux")
            nc.sync.dma_start(out=aux_sb[:], in_=aux[:])
            wr_sb = wpool.tile([P, HID], F32, tag="wr")
            nc.scalar.dma_start(out=wr_sb[:], in_=wr[:])

            hid_col = aux_sb[:, 0:1]  # [128, 1] hidden slice
            ident = aux_sb[:, 1 : 1 + P]  # [128, 128] identity
            ones_row = aux_sb[0:1, 1 + P : 1 + 2 * P]  # [1, 128] of 1.0

            # ---- enc chunks: all issued up-front, alternating HWDGE rings --
            enc_ts = []
            for c in range(NCH):
                t = encp.tile([P, CU, HID], F32, name="enc_t")
                eng = nc.sync if c % 2 == 0 else nc.scalar
                eng.dma_start(out=t[:], in_=enc_v[c])
                enc_ts.append(t)

            # ---- partial v on PE: vp[1, h] = sum_o hidden[o] * wr[o, h] ----
            vp_ps = [
                ps_a.tile([1, 512], F32, tag=f"vp{n}", name=f"vp_ps{n}")
                for n in range(2)
            ]
            ccv_sb = small.tile([1, HID], F32, tag="ccv")
            for n in range(2):
                nc.tensor.matmul(
                    vp_ps[n][:],
                    hid_col,
                    wr_sb[:, n * 512 : (n + 1) * 512],
                    start=True,
                    stop=True,
                )
                nc.vector.tensor_copy(
                    ccv_sb[:, n * 512 : (n + 1) * 512], vp_ps[n][:]
                )

            # ---- AllReduce(add) the partial v (also warms the CC stream) ---
            nc.gpsimd.dma_start(out=ccv_in[:], in_=ccv_sb[:])
            nc.gpsimd.collective_compute(
                "AllReduce",
                AL.add,
                replica_groups=[list(range(NCORES))],
                ins=[ccv_in.ap().opt()],
                outs=[ccv_out.ap().opt()],
            )
            v_sb = small.tile([1, HID], F32, tag="vsb")
            nc.gpsimd.dma_start(out=v_sb[:], in_=ccv_out[:])

            # ---- broadcast v to all partitions via PE ones-outer-product ---
            vb = small.tile([P, HID], F32, tag="vb")
            vb_ps = [
                ps_b.tile([P, 512], F32, tag=f"vb{n}", name=f"vb_ps{n}")
                for n in range(2)
            ]
            for n in range(2):
                nc.tensor.matmul(
                    vb_ps[n][:],
                    ones_row,
                    v_sb[:, n * 512 : (n + 1) * 512],
                    start=True,
                    stop=True,
                )
            nc.vector.tensor_copy(vb[:, 0:512], vb_ps[0][:])
            nc.scalar.copy(vb[:, 512:1024], vb_ps[1][:])

            # ---- energies: E[p, c*4+u] = enc_row . v ----------------------
            # rows 0,1 on DVE: fused mult+accum (scalar_tensor_tensor) with a
            # broadcast dummy out -> no product write-back traffic.
            # rows 2,3: GpSimd mult -> ACT Copy+accum reduce.
            e_sb = small.tile([P, TW], F32, tag="e")
            jd = jdp.tile([P, 1], F32, tag="jd_dummy")
            for c in range(NCH):
                enc_t = enc_ts[c]
                base = c * CU
                for u in (0, 1):
                    nc.vector.scalar_tensor_tensor(
                        out=jd.broadcast_to(enc_t[:, u, :].shape),
                        in0=enc_t[:, u, :],
                        scalar=1.0,
                        in1=vb[:],
                        op0=AL.mult,
                        op1=AL.mult,
                        accum_out=e_sb[:, base + u : base + u + 1],
                    )
                for u in (2, 3):
                    jg = jgp.tile([P, HID], F32, name="jg")
                    nc.gpsimd.tensor_tensor(
                        out=jg[:],
                        in0=enc_t[:, u, :],
                        in1=vb[:],
                        op=AL.mult,
                    )
                    nc.scalar.activation(
                        jg[:],
                        jg[:],
                        ACT.Copy,
                        accum_out=e_sb[:, base + u : base + u + 1],
                    )

            # ---- local per-partition softmax stats ------------------------
            # ms[:,0] = -m_p (negated row max), ms[:,1] = s_p = sum exp(e-m_p)
            ms = small.tile([P, 2], F32, tag="ms")
            m_sb = small.tile([P, 1], F32, tag="m")
            nc.vector.tensor_reduce(
                m_sb[:], e_sb[:], axis=mybir.AxisListType.X, op=AL.max
            )
            nc.vector.tensor_scalar(
                out=ms[:, 0:1],
                in0=m_sb[:],
                scalar1=-1.0,
                scalar2=None,
                op0=AL.mult,
            )
            eexp = small.tile([P, TW], F32, tag="eexp")
            nc.scalar.activation(
                eexp[:],
                e_sb[:],
                ACT.Exp,
                bias=ms[:, 0:1],
                accum_out=ms[:, 1:2],
            )

            # ---- exchange per-partition stats: [2,128] x 8 cores ----------
            tr_ps = ps_c.tile([2, P], F32, tag="tr", name="tr_ps")
            nc.tensor.transpose(tr_ps[:], ms[:], ident)
            cc_sb = small.tile([2, P], F32, tag="ccs")
            nc.vector.tensor_copy(cc_sb[:], tr_ps[:])
            nc.gpsimd.dma_start(out=cc_in[:], in_=cc_sb[:])
            nc.gpsimd.collective_compute(
                "AllGather",
                AL.bypass,
                replica_groups=[list(range(NCORES))],
                ins=[cc_in.ap().opt()],
                outs=[cc_out.ap().opt()],
            )
            ag_sb = small.tile([1, 2 * NCORES * P], F32, tag="ag")
            nc.gpsimd.dma_start(
                out=ag_sb[:], in_=cc_out.rearrange("a b -> (a b)")
            )
            ag4 = ag_sb[:].rearrange(
                "p (r two h) -> p r two h", r=NCORES, two=2
            )
            nm_all = ag4[:, :, 0, :]  # [1, 8, 128] of -m_rp
            s_all = ag4[:, :, 1, :]  # [1, 8, 128] of s_rp

            # g2[:,0] = gnm = min(-m) = -M ; g2[:,1] = 1/gsum
            g2 = small.tile([1, 2], F32, tag="g2")
            nc.vector.tensor_reduce(
                g2[:, 0:1], nm_all, axis=mybir.AxisListType.XY, op=AL.min
            )
            diffs = small.tile([1, NCORES, P], F32, tag="diffs")
            nc.vector.tensor_scalar(
                out=diffs[:],
                in0=nm_all,
                scalar1=g2[:, 0:1],
                scalar2=None,
                op0=AL.subtract,
            )
            # edifs = exp(-(nm - gnm)) = exp(m_rp - M)
            edifs = small.tile([1, NCORES, P], F32, tag="edifs")
            nc.scalar.activation(edifs[:], diffs[:], ACT.Exp, scale=-1.0)
            # gsum = sum s_rp * exp(m_rp - M), fused mult+accum
            jd2 = small.tile([1, 1], F32, tag="jd2_dummy")
            gsum = small.tile([1, 1], F32, tag="gsum")
            nc.vector.scalar_tensor_tensor(
                out=jd2.broadcast_to(edifs[:].shape),
                in0=edifs[:],
                scalar=1.0,
                in1=s_all,
                op0=AL.mult,
                op1=AL.mult,
                accum_out=gsum[:],
            )
            nc.vector.reciprocal(g2[:, 1:2], gsum[:])

            # ---- broadcast (gnm, 1/gsum) to all partitions, rescale -------
            bc_ps = ps_c.tile([P, 2], F32, tag="bc", name="bc_ps")
            nc.tensor.matmul(bc_ps[:], ones_row, g2[:], start=True, stop=True)
            bc = small.tile([P, 2], F32, tag="bc_sb")
            nc.scalar.copy(bc[:], bc_ps[:])
            d_p = small.tile([P, 1], F32, tag="dp")
            nc.vector.tensor_tensor(
                out=d_p[:], in0=ms[:, 0:1], in1=bc[:, 0:1], op=AL.subtract
            )
            # f0 = exp(-(nm_p - gnm)) = exp(m_p - M)
            f0 = small.tile([P, 1], F32, tag="f0")
            nc.scalar.activation(f0[:], d_p[:], ACT.Exp, scale=-1.0)
            f = small.tile([P, 1], F32, tag="f")
            nc.vector.tensor_tensor(
                out=f[:], in0=f0[:], in1=bc[:, 1:2], op=AL.mult
            )

            # ---- attn = eexp * f, store -----------------------------------
            attn_sb = small.tile([P, TW], F32, tag="attn")
            nc.scalar.mul(attn_sb[:], eexp[:], f[:])
            nc.sync.dma_start(out=out_v, in_=attn_sb[:])

    _split_multiwaits(nc)
    return nc


def _get_nc():
    if "nc" not in _CACHE:
        _CACHE["nc"] = _build_nc()
    return _CACHE["nc"]


def _prep_in_maps(hidden, encoder_outputs, W, b):
    hidden = np.ascontiguousarray(np.asarray(hidden, dtype=np.float32))
    enc = np.ascontiguousarray(np.asarray(encoder_outputs, dtype=np.float32))
    W = np.ascontiguousarray(np.asarray(W, dtype=np.float32))
    ident = np.eye(P, dtype=np.float32)
    ones = np.ones((P, P), dtype=np.float32)
    in_maps = []
    for c in range(NCORES):
        hid_col = hidden[c * P : (c + 1) * P].reshape(P, 1)
        auxc = np.ascontiguousarray(
            np.concatenate([hid_col, ident, ones], axis=1), dtype=np.float32
        )
        in_maps.append(
            {
                "enc": enc[c * SHARD : (c + 1) * SHARD],
                "wr": np.ascontiguousarray(W[c * P : (c + 1) * P, :]),
                "aux": auxc,
            }
        )
    return in_maps


def _ensure_ntff_hook():
    """Register the axon NTFF profile hook that this deployment's antenv
    package is missing, so trace=True yields a real HW profile."""
    import sys as _sys
    import types

    if "antenv.axon_hooks" in _sys.modules:
        return
    mod = types.ModuleType("antenv.axon_hooks")
    holder = [None]
    mod.set_axon_ntff_profile_hook = lambda h: holder.__setitem__(0, h)
    mod.get_axon_ntff_profile_hook = lambda: holder[0]
    _sys.modules["antenv.axon_hooks"] = mod
    import antenv

    antenv.axon_hooks = mod
    try:
        if "/root/.axon_site" not in _sys.path:
            _sys.path.insert(0, "/root/.axon_site")
        from trn_agent_boot.trn_boot import _ntff_profile_via_ctypes

        hook = _ntff_profile_via_ctypes("/opt/axon/libaxon_pjrt.so")
        if hook is not None:
            mod.set_axon_ntff_profile_hook(hook)
    except Exception as e:  # degrade to no tracing
        print(f"ntff hook registration failed: {e}", file=_sys.stderr)
    # artifact upload needs no external bucket for local profiling
    from concourse import bass_utils as _bu

    _bu.upload_artifacts = lambda tmpdir: tmpdir


def run(hidden, encoder_outputs, W, b, trace=False, **trace_kw):
    if trace:
        _ensure_ntff_hook()
    nc = _get_nc()
    in_maps = _prep_in_maps(hidden, encoder_outputs, W, b)
    res = run_bass_kernel_spmd(
        nc, in_maps, list(range(NCORES)), trace=trace, **trace_kw
    )
    shards = [np.asarray(res.results[c]["attn"]) for c in range(NCORES)]
    full = np.concatenate(shards).astype(np.float32)
    return full[None, None, :], res


def kernel(hidden, encoder_outputs, W, b):
    out, _ = run(hidden, encoder_outputs, W, b, trace=False)
    return out


# revision 7
# speedup vs baseline: 1.2697x; 1.2697x over previous
"""Bass/Trainium2 kernel for nn_Attn_81690277970335.

reference:  proj = enc @ W.T + b        [S, H]
            energies = proj @ hidden    [S]
            attn = softmax(energies)    [1, 1, S]

Key algebraic identity (exact in exact arithmetic):
            energies = enc @ (W.T @ hidden) + (b . hidden)
and softmax is invariant to the constant shift (b . hidden).  So on device we
compute v = W.T @ hidden once (small matvec) and then a single [S,H] @ [H]
matvec over the big tensor -> memory-bound streaming of enc.

Distribution: encoder_outputs sharded along seq across 8 cores; W, hidden
replicated (the first collective of an execution only reaches its mesh phase
~80us in regardless of trigger time, so v must NOT depend on a collective;
a dummy AllGather fired at kernel start absorbs that init cost so the real
stats AllGather at the end is prompt).

DMA: W (4.2 MB) split across both HWDGE rings first (v gates all compute),
then the 8 x 2MB enc chunks, all issued up-front (whole shard fits in SBUF,
no DMA ever waits on compute).  Tail DMAs ride the by-then-idle sync ring.

Dot products: per chunk of 4 seq-rows/partition, 3 rows on DVE (fused
mult+accum scalar_tensor_tensor with a broadcast dummy out -> no product
write-back), 1 row on GpSimd (plain mult) reduced by ACT (Copy+accum).
Measured rates: DVE ~1.6us, GpSimd ~3.3us, ACT ~1.4us per [128,1024] row.

Softmax: per-partition max m_p / sum s_p (no cross-partition work before the
collective); one AllGather of packed [2,128] (-m_p, s_p) stats; every core
redundantly combines all 8*128 pairs and rescales its exp(e - m_p) tile.
"""

import sys

sys.path.insert(0, "/opt/trn_rl_repo")

import numpy as np

import concourse.bass as bass
import concourse.mybir as mybir
import concourse.tile as tile
from concourse.bass_utils import run_bass_kernel_spmd

SEQ = 32768
HID = 1024
NCORES = 8
SHARD = SEQ // NCORES  # 4096
P = 128  # partitions
TW = SHARD // P  # 32 seq rows per partition
CU = 4  # rows per chunk (per partition)
NCH = TW // CU  # 8 chunks
KCH = HID // P  # 8 contraction chunks for v
F32 = mybir.dt.float32
AL = mybir.AluOpType
ACT = mybir.ActivationFunctionType

_CACHE = {}


def _split_multiwaits(nc):
    """This container's walrus build accepts at most ONE sync-wait per
    instruction; Tile emits several.  Hoist extra waits onto single-wait
    NoOps inserted just before the instruction on the same engine queue
    (engines and DGE-issuing sequencers are in-order, so semantics hold)."""
    import bass_rust

    cnt = 0
    for f in nc.m.functions:
        for bb in f.blocks:
            il = bb.instructions
            i = 0
            while i < len(il):
                inst = il[i]
                si = inst.sync_info
                if si is not None and si.on_wait and len(si.on_wait) > 1:
                    waits = list(si.on_wait)
                    keep, extra = waits[-1], waits[:-1]
                    for j, w in enumerate(extra):
                        nop = mybir.InstNoOp(
                            name=f"{inst.name}-w{j}", ins=[], outs=[]
                        )
                        nop.engine = inst.engine
                        nop.sync_info = bass_rust.SyncInfo(
                            on_wait=[w], on_update=[]
                        )
                        il.insert(i, nop)
                        i += 1
                        cnt += 1
                    inst.sync_info = bass_rust.SyncInfo(
                        on_wait=[keep], on_update=list(si.on_update or [])
                    )
                i += 1
    return cnt


def _build_nc():
    nc = bass.Bass(num_devices=NCORES)

    enc = nc.dram_tensor("enc", [SHARD, HID], F32, kind="ExternalInput")
    # full W, host-restaged so o-chunk k, row p = W[k*128+p, :]:
    # wt[p, k, h] = W[k*128+p, h]
    wt = nc.dram_tensor("wt", [P, KCH, HID], F32, kind="ExternalInput")
    # aux: [128, 8 + 128 + 128]: hid_pk | ident | ones
    AUXW = KCH + P + P
    aux = nc.dram_tensor("aux", [P, AUXW], F32, kind="ExternalInput")
    out = nc.dram_tensor("attn", [SHARD], F32, kind="ExternalOutput")

    # collective bounce buffers (internal DRAM; outputs must be Shared)
    dummy_in = nc.dram_tensor("dummy_in", [1, 1], F32)
    dummy_out = nc.dram_tensor("dummy_out", [NCORES, 1], F32, addr_space="Shared")
    cc_in = nc.dram_tensor("cc_in", [2, P], F32)
    cc_out = nc.dram_tensor("cc_out", [2 * NCORES, P], F32, addr_space="Shared")

    # seq row s of the shard lives at (partition p, column t): s = p*TW + t,
    # t = c*CU + u.  Per chunk c: 4 consecutive rows -> 16KB contiguous per
    # partition -> line-rate descriptors.
    enc_v = enc.rearrange("(p c u) h -> c p u h", c=NCH, u=CU)  # [8,128,4,HID]
    out_v = out.rearrange("(p t) -> p t", t=TW)  # [128, TW]

    with tile.TileContext(nc) as tc:
        with (
            tc.tile_pool(name="wpool", bufs=1) as wpool,
            tc.tile_pool(name="encp", bufs=NCH) as encp,
            tc.tile_pool(name="jg", bufs=2) as jgp,
            tc.tile_pool(name="small", bufs=1) as small,
            tc.tile_pool(name="ps_v", bufs=1, space="PSUM") as ps_v,
            tc.tile_pool(name="ps_c", bufs=1, space="PSUM") as ps_c,
        ):
            # ---- dummy collective at queue head: absorbs the ~80us CC
            # stream init so the real AllGather at the end is prompt --------
            nc.gpsimd.collective_compute(
                "AllGather",
                AL.bypass,
                replica_groups=[list(range(NCORES))],
                ins=[dummy_in.ap().opt()],
                outs=[dummy_out.ap().opt()],
            )

            # ---- W first on both rings (v gates all compute), then aux ----
            wa = wpool.tile([P, KCH // 2, HID], F32, tag="wa")
            wb = wpool.tile([P, KCH // 2, HID], F32, tag="wb")
            nc.sync.dma_start(out=wa[:], in_=wt[:, 0 : KCH // 2, :])
            nc.scalar.dma_start(out=wb[:], in_=wt[:, KCH // 2 : KCH, :])
            aux_sb = wpool.tile([P, AUXW], F32, tag="aux")
            nc.sync.dma_start(out=aux_sb[:], in_=aux[:])

            hid_pk = aux_sb[:, 0:KCH]  # [128, 8] hidden o-chunks
            ident = aux_sb[:, KCH : KCH + P]  # [128, 128] identity
            ones_row = aux_sb[0:1, KCH + P : KCH + 2 * P]  # [1, 128] of 1.0

            # ---- enc chunks: all issued up-front, alternating HWDGE rings --
            enc_ts = []
            for c in range(NCH):
                t = encp.tile([P, CU, HID], F32, name="enc_t")
                eng = nc.sync if c % 2 == 0 else nc.scalar
                eng.dma_start(out=t[:], in_=enc_v[c])
                enc_ts.append(t)

            # ---- v = W.T @ hidden, replicated on all partitions -----------
            # stationary = hidden o-chunk broadcast into all 128 PE columns
            # -> accumulated result lands replicated, no broadcast pass.
            # k-chunks 0-3 read wa, 4-7 read wb, so PE starts after wa only.
            vb_ps = [
                ps_v.tile([P, 512], F32, tag=f"vb{n}", name=f"vb_ps{n}")
                for n in range(2)
            ]
            for n in range(2):
                for k in range(KCH):
                    w_sb = wa if k < KCH // 2 else wb
                    kk = k % (KCH // 2)
                    nc.tensor.matmul(
                        vb_ps[n][:],
                        hid_pk[:, k : k + 1].broadcast_to([P, P]),
                        w_sb[:, kk, n * 512 : (n + 1) * 512],
                        start=(k == 0),
                        stop=(k == KCH - 1),
                    )
            vb = small.tile([P, HID], F32, tag="vb")
            nc.vector.tensor_copy(vb[:, 0:512], vb_ps[0][:])
            nc.scalar.copy(vb[:, 512:1024], vb_ps[1][:])

            # ---- energies: E[p, c*4+u] = enc_row . v ----------------------
            # rows 0-2 on DVE: fused mult+accum with broadcast dummy out.
            # row 3: GpSimd mult -> ACT Copy+accum reduce.
            e_sb = small.tile([P, TW], F32, tag="e")
            jd = small.tile([P, 1], F32, tag="jd_dummy")
            for c in range(NCH):
                enc_t = enc_ts[c]
                base = c * CU
                for u in (0, 1, 2):
                    nc.vector.scalar_tensor_tensor(
                        out=jd.broadcast_to(enc_t[:, u, :].shape),
                        in0=enc_t[:, u, :],
                        scalar=1.0,
                        in1=vb[:],
                        op0=AL.mult,
                        op1=AL.mult,
                        accum_out=e_sb[:, base + u : base + u + 1],
                    )
                u = 3
                jg = jgp.tile([P, HID], F32, name="jg")
                nc.gpsimd.tensor_tensor(
                    out=jg[:], in0=enc_t[:, u, :], in1=vb[:], op=AL.mult
                )
                nc.scalar.activation(
                    jg[:],
                    jg[:],
                    ACT.Copy,
                    accum_out=e_sb[:, base + u : base + u + 1],
                )

            # ---- local per-partition softmax stats ------------------------
            # ms[:,0] = -m_p (negated row max), ms[:,1] = s_p = sum exp(e-m_p)
            ms = small.tile([P, 2], F32, tag="ms")
            m_sb = small.tile([P, 1], F32, tag="m")
            nc.vector.tensor_reduce(
                m_sb[:], e_sb[:], axis=mybir.AxisListType.X, op=AL.max
            )
            nc.vector.tensor_scalar(
                out=ms[:, 0:1],
                in0=m_sb[:],
                scalar1=-1.0,
                scalar2=None,
                op0=AL.mult,
            )
            eexp = small.tile([P, TW], F32, tag="eexp")
            nc.scalar.activation(
                eexp[:],
                e_sb[:],
                ACT.Exp,
                bias=ms[:, 0:1],
                accum_out=ms[:, 1:2],
            )

            # ---- exchange per-partition stats: [2,128] x 8 cores ----------
            tr_ps = ps_c.tile([2, P], F32, tag="tr", name="tr_ps")
            nc.tensor.transpose(tr_ps[:], ms[:], ident)
            cc_sb = small.tile([2, P], F32, tag="ccs")
            nc.vector.tensor_copy(cc_sb[:], tr_ps[:])
            nc.sync.dma_start(out=cc_in[:], in_=cc_sb[:])
            nc.gpsimd.collective_compute(
                "AllGather",
                AL.bypass,
                replica_groups=[list(range(NCORES))],
                ins=[cc_in.ap().opt()],
                outs=[cc_out.ap().opt()],
            )
            ag_sb = small.tile([1, 2 * NCORES * P], F32, tag="ag")
            nc.sync.dma_start(
                out=ag_sb[:], in_=cc_out.rearrange("a b -> (a b)")
            )
            ag4 = ag_sb[:].rearrange(
                "p (r two h) -> p r two h", r=NCORES, two=2
            )
            nm_all = ag4[:, :, 0, :]  # [1, 8, 128] of -m_rp
            s_all = ag4[:, :, 1, :]  # [1, 8, 128] of s_rp

            # g2[:,0] = gnm = min(-m) = -M ; g2[:,1] = 1/gsum
            g2 = small.tile([1, 2], F32, tag="g2")
            nc.vector.tensor_reduce(
                g2[:, 0:1], nm_all, axis=mybir.AxisListType.XY, op=AL.min
            )
            diffs = small.tile([1, NCORES, P], F32, tag="diffs")
            nc.vector.tensor_scalar(
                out=diffs[:],
                in0=nm_all,
                scalar1=g2[:, 0:1],
                scalar2=None,
                op0=AL.subtract,
            )
            # edifs = exp(-(nm - gnm)) = exp(m_rp - M)
            edifs = small.tile([1, NCORES, P], F32, tag="edifs")
            nc.scalar.activation(edifs[:], diffs[:], ACT.Exp, scale=-1.0)
            # gsum = sum s_rp * exp(m_rp - M), fused mult+accum
            jd2 = small.tile([1, 1], F32, tag="jd2_dummy")
            gsum = small.tile([1, 1], F32, tag="gsum")
            nc.vector.scalar_tensor_tensor(
                out=jd2.broadcast_to(edifs[:].shape),
                in0=edifs[:],
                scalar=1.0,
                in1=s_all,
                op0=AL.mult,
                op1=AL.mult,
                accum_out=gsum[:],
            )
            nc.vector.reciprocal(g2[:, 1:2], gsum[:])

            # ---- broadcast (gnm, 1/gsum) to all partitions, rescale -------
            bc_ps = ps_c.tile([P, 2], F32, tag="bc", name="bc_ps")
            nc.tensor.matmul(bc_ps[:], ones_row, g2[:], start=True, stop=True)
            d_p = small.tile([P, 1], F32, tag="dp")
            nc.vector.tensor_tensor(
                out=d_p[:], in0=ms[:, 0:1], in1=bc_ps[:, 0:1], op=AL.subtract
            )
            # f0 = exp(-(nm_p - gnm)) = exp(m_p - M)
            f0 = small.tile([P, 1], F32, tag="f0")
            nc.scalar.activation(f0[:], d_p[:], ACT.Exp, scale=-1.0)
            f = small.tile([P, 1], F32, tag="f")
            nc.vector.tensor_tensor(
                out=f[:], in0=f0[:], in1=bc_ps[:, 1:2], op=AL.mult
            )

            # ---- attn = eexp * f, store -----------------------------------
            attn_sb = small.tile([P, TW], F32, tag="attn")
            nc.scalar.mul(attn_sb[:], eexp[:], f[:])
            nc.sync.dma_start(out=out_v, in_=attn_sb[:])

    _split_multiwaits(nc)
    return nc


def _get_nc():
    if "nc" not in _CACHE:
        _CACHE["nc"] = _build_nc()
    return _CACHE["nc"]


def _prep_in_maps(hidden, encoder_outputs, W, b):
    hidden = np.ascontiguousarray(np.asarray(hidden, dtype=np.float32))
    enc = np.ascontiguousarray(np.asarray(encoder_outputs, dtype=np.float32))
    W = np.ascontiguousarray(np.asarray(W, dtype=np.float32))
    # wt[p, k, h] = W[k*128+p, h]
    wt = np.ascontiguousarray(
        W.reshape(KCH, P, HID).transpose(1, 0, 2)
    )
    hid_pk = hidden.reshape(KCH, P).T  # [128, 8]
    ident = np.eye(P, dtype=np.float32)
    ones = np.ones((P, P), dtype=np.float32)
    auxc = np.ascontiguousarray(
        np.concatenate([hid_pk, ident, ones], axis=1), dtype=np.float32
    )
    in_maps = []
    for c in range(NCORES):
        in_maps.append(
            {
                "enc": enc[c * SHARD : (c + 1) * SHARD],
                "wt": wt,
                "aux": auxc,
            }
        )
    return in_maps


def _ensure_ntff_hook():
    """Register the axon NTFF profile hook that this deployment's antenv
    package is missing, so trace=True yields a real HW profile."""
    import sys as _sys
    import types

    if "antenv.axon_hooks" in _sys.modules:
        return
    mod = types.ModuleType("antenv.axon_hooks")
    holder = [None]
    mod.set_axon_ntff_profile_hook = lambda h: holder.__setitem__(0, h)
    mod.get_axon_ntff_profile_hook = lambda: holder[0]
    _sys.modules["antenv.axon_hooks"] = mod
    import antenv

    antenv.axon_hooks = mod
    try:
        if "/root/.axon_site" not in _sys.path:
            _sys.path.insert(0, "/root/.axon_site")
        from trn_agent_boot.trn_boot import _ntff_profile_via_ctypes

        hook = _ntff_profile_via_ctypes("/opt/axon/libaxon_pjrt.so")
        if hook is not None:
            mod.set_axon_ntff_profile_hook(hook)
    except Exception as e:  # degrade to no tracing
        print(f"ntff hook registration failed: {e}", file=_sys.stderr)
    # artifact upload needs no external bucket for local profiling
    from concourse import bass_utils as _bu

    _bu.upload_artifacts = lambda tmpdir: tmpdir


def run(hidden, encoder_outputs, W, b, trace=False, **trace_kw):
    if trace:
        _ensure_ntff_hook()
    nc = _get_nc()
    in_maps = _prep_in_maps(hidden, encoder_outputs, W, b)
    res = run_bass_kernel_spmd(
        nc, in_maps, list(range(NCORES)), trace=trace, **trace_kw
    )
    shards = [np.asarray(res.results[c]["attn"]) for c in range(NCORES)]
    full = np.concatenate(shards).astype(np.float32)
    return full[None, None, :], res


def kernel(hidden, encoder_outputs, W, b):
    out, _ = run(hidden, encoder_outputs, W, b, trace=False)
    return out
